# revision 5
# baseline (speedup 1.0000x reference)
"""Trainium2 Bass kernel for CustomAttention (B=4, F=T=2048, D=768, N=12, H=64).

Returns (context [B,F,N,H] f32, scores [B,N,F,T] f32) where scores are the
masked, scaled pre-softmax logits.

Sharding: 8 cores = batch(4) x F-half(2). Each core computes, for its
(batch, F-half) slice, all 12 heads:
  - qT [H, F_loc], kT [H, T] projections (head dim on partitions) from
    host-pre-transposed fromT/toT, with 1/sqrt(H) folded into Wq/bq.
  - v [T, H] projections with an appended ones column (v_aug [T, H+1]).
  - per head: scoresT[t, f] = kT.T-style matmul -> +adderT (mask) on DVE
    (this fp32 tile is the scores output, written transposed; host
    transposes back) -> exp on ACT (bf16 probsT) -> context matmul
    lhsT=v_aug, rhs=probsT accumulating over t, giving
    ctx_t [H+1, F_loc]: rows 0..63 = unnormalized context, row 64 = the
    softmax denominator. Host divides, adds bv, reassembles.
"""

import sys

if "/opt/trn_rl_repo" not in sys.path:
    sys.path.insert(0, "/opt/trn_rl_repo")

import numpy as np
import ml_dtypes

import concourse.bass as bass  # noqa: F401  (engine types resolve through nc)
import concourse.mybir as mybir
import concourse.tile as tile
from concourse import bacc
from concourse.bass_utils import run_bass_kernel_spmd

B, F, T, D = 4, 2048, 2048, 768
N, H = 12, 64
NCORES = 8
FS = F // 2          # F rows per core
NPAIR = N // 2       # head pairs (2 heads packed per 128 partitions)
DT6 = D // 128       # contraction tiles over D
TT = T // 128        # t tiles
FC = FS // 512       # 512-wide moving chunks over F_loc
SCALE = 1.0 / 8.0    # 1/sqrt(H)

MM_DT = mybir.dt.bfloat16
NP_MM = ml_dtypes.bfloat16
F32 = mybir.dt.float32

_cache: dict = {}


def _build_nc():
    nc = bacc.Bacc("TRN2", target_bir_lowering=False, debug=False,
                   num_devices=NCORES)
    fromT = nc.dram_tensor("fromT", [D, FS], MM_DT, kind="ExternalInput")
    toT = nc.dram_tensor("toT", [D, T], MM_DT, kind="ExternalInput")
    wq = nc.dram_tensor("wq", [D, N * H], MM_DT, kind="ExternalInput")
    wk = nc.dram_tensor("wk", [D, N * H], MM_DT, kind="ExternalInput")
    wv = nc.dram_tensor("wv", [D, N * H], MM_DT, kind="ExternalInput")
    bqv = nc.dram_tensor("bqv", [N * H], F32, kind="ExternalInput")
    bkv = nc.dram_tensor("bkv", [N * H], F32, kind="ExternalInput")
    adderT = nc.dram_tensor("adderT", [T, FS], F32, kind="ExternalInput")
    scores_t = nc.dram_tensor("scores_t", [N, T, FS], F32,
                              kind="ExternalOutput")
    ctx_t = nc.dram_tensor("ctx_t", [N, H + 1, FS], F32,
                           kind="ExternalOutput")

    ExpF = mybir.ActivationFunctionType.Exp
    CopyF = mybir.ActivationFunctionType.Copy
    IdentF = mybir.ActivationFunctionType.Identity

    with tile.TileContext(nc) as tc:
        with (
            tc.tile_pool(name="persist", bufs=1) as pp,
            tc.tile_pool(name="consts", bufs=1) as cst,
        ):
            # Persistent SBUF residents.
            qt_all = pp.tile([128, NPAIR, FS], MM_DT)     # qT pair-packed
            kt_all = pp.tile([128, NPAIR, T], MM_DT)      # kT pair-packed
            v_all = pp.tile([128, N, TT, H + 1], MM_DT)   # v_aug per head/tt
            adder_sb = pp.tile([128, TT, FS], F32)        # mask adder (T,F)

            bq_sb = cst.tile([128, NPAIR], F32)
            bk_sb = cst.tile([128, NPAIR], F32)

            nc.sync.dma_start(
                out=adder_sb,
                in_=adderT.ap().rearrange("(tt p) f -> p tt f", p=128))
            nc.sync.dma_start(
                out=bq_sb, in_=bqv.ap().rearrange("(n p) -> p n", p=128))
            nc.sync.dma_start(
                out=bk_sb, in_=bkv.ap().rearrange("(n p) -> p n", p=128))
            nc.vector.memset(v_all[:, :, :, H:H + 1], 1.0)

            # ---------------- Phase 1: projections ----------------
            with (
                tc.tile_pool(name="p1", bufs=1) as p1,
                tc.tile_pool(name="ppsum", bufs=4, space="PSUM") as ppsum,
            ):
                from_sb = p1.tile([128, DT6, FS], MM_DT)
                to_sb = p1.tile([128, DT6, T], MM_DT)
                wq_sb = p1.tile([128, DT6, N * H], MM_DT)
                wk_sb = p1.tile([128, DT6, N * H], MM_DT)
                wv_sb = p1.tile([128, DT6, N * H], MM_DT)
                nc.sync.dma_start(
                    out=from_sb,
                    in_=fromT.ap().rearrange("(n p) f -> p n f", p=128))
                nc.sync.dma_start(
                    out=to_sb,
                    in_=toT.ap().rearrange("(n p) t -> p n t", p=128))
                nc.sync.dma_start(
                    out=wq_sb,
                    in_=wq.ap().rearrange("(n p) h -> p n h", p=128))
                nc.sync.dma_start(
                    out=wk_sb,
                    in_=wk.ap().rearrange("(n p) h -> p n h", p=128))
                nc.sync.dma_start(
                    out=wv_sb,
                    in_=wv.ap().rearrange("(n p) h -> p n h", p=128))

                # qT / kT: out partitions = 2 heads x 64, moving = F or T.
                for pair in range(NPAIR):
                    hs = slice(pair * 128, (pair + 1) * 128)
                    for fc in range(FC):
                        ps = ppsum.tile([128, 512], F32, tag="ps")
                        for d6 in range(DT6):
                            nc.tensor.matmul(
                                ps, lhsT=wq_sb[:, d6, hs],
                                rhs=from_sb[:, d6, fc * 512:(fc + 1) * 512],
                                start=(d6 == 0), stop=(d6 == DT6 - 1))
                        nc.scalar.activation(
                            qt_all[:, pair, fc * 512:(fc + 1) * 512], ps,
                            IdentF, bias=bq_sb[:, pair:pair + 1])
                    for tc4 in range(T // 512):
                        ps = ppsum.tile([128, 512], F32, tag="ps")
                        for d6 in range(DT6):
                            nc.tensor.matmul(
                                ps, lhsT=wk_sb[:, d6, hs],
                                rhs=to_sb[:, d6, tc4 * 512:(tc4 + 1) * 512],
                                start=(d6 == 0), stop=(d6 == DT6 - 1))
                        nc.scalar.activation(
                            kt_all[:, pair, tc4 * 512:(tc4 + 1) * 512], ps,
                            IdentF, bias=bk_sb[:, pair:pair + 1])

                # v: out partitions = t rows, moving = 6 heads x 64.
                for tt in range(TT):
                    ts = slice(tt * 128, (tt + 1) * 128)
                    for hg in range(2):
                        ps = ppsum.tile([128, 384], F32, tag="psv")
                        for d6 in range(DT6):
                            nc.tensor.matmul(
                                ps, lhsT=to_sb[:, d6, ts],
                                rhs=wv_sb[:, d6, hg * 384:(hg + 1) * 384],
                                start=(d6 == 0), stop=(d6 == DT6 - 1))
                        for h6 in range(6):
                            h = hg * 6 + h6
                            nc.scalar.activation(
                                v_all[:, h, tt, 0:H],
                                ps[:, h6 * 64:(h6 + 1) * 64], CopyF)

            # ---------------- Phase 2: scores + context ----------------
            with (
                tc.tile_pool(name="work", bufs=4) as wk_pool,
                tc.tile_pool(name="spsum", bufs=4, space="PSUM") as spsum,
                tc.tile_pool(name="cpsum", bufs=2, space="PSUM") as cpsum,
            ):
                for h in range(N):
                    pair, sub = h // 2, h % 2
                    hp = slice(sub * 64, (sub + 1) * 64)
                    ctx_ps = [cpsum.tile([H + 1, 512], F32, tag=f"ctx{fc}",
                                         name=f"ctx_ps{fc}_{h}")
                              for fc in range(FC)]
                    for tt in range(TT):
                        sT = wk_pool.tile([128, FS], F32, tag="sT")
                        pT = wk_pool.tile([128, FS], MM_DT, tag="pT")
                        for fc in range(FC):
                            fsl = slice(fc * 512, (fc + 1) * 512)
                            ps = spsum.tile([128, 512], F32, tag="ps")
                            nc.tensor.matmul(
                                ps,
                                lhsT=kt_all[hp, pair,
                                            tt * 128:(tt + 1) * 128],
                                rhs=qt_all[hp, pair, fsl],
                                start=True, stop=True)
                            nc.vector.tensor_add(
                                sT[:, fsl], ps, adder_sb[:, tt, fsl])
                            nc.scalar.activation(pT[:, fsl], sT[:, fsl], ExpF)
                            nc.tensor.matmul(
                                ctx_ps[fc], lhsT=v_all[:, h, tt, :],
                                rhs=pT[:, fsl],
                                start=(tt == 0), stop=(tt == TT - 1))
                        nc.sync.dma_start(
                            out=scores_t.ap()[h, tt * 128:(tt + 1) * 128, :],
                            in_=sT)
                    ctx_sb = wk_pool.tile([H + 1, FS], F32, tag="ctx_sb",
                                          name=f"ctx_sb_{h}")
                    for fc in range(FC):
                        nc.scalar.activation(
                            ctx_sb[:, fc * 512:(fc + 1) * 512], ctx_ps[fc],
                            CopyF)
                    nc.sync.dma_start(out=ctx_t.ap()[h, :, :], in_=ctx_sb)

    nc.compile()
    return nc


def kernel(from_tensor, to_tensor, attention_mask, Wq, Wk, Wv, bq, bk, bv):
    from_tensor = np.asarray(from_tensor, dtype=np.float32)
    to_tensor = np.asarray(to_tensor, dtype=np.float32)
    attention_mask = np.asarray(attention_mask)
    Wq = np.asarray(Wq, dtype=np.float32)
    Wk = np.asarray(Wk, dtype=np.float32)
    Wv = np.asarray(Wv, dtype=np.float32)
    bq = np.asarray(bq, dtype=np.float32)
    bk = np.asarray(bk, dtype=np.float32)
    bv = np.asarray(bv, dtype=np.float32)

    if "nc" not in _cache:
        _cache["nc"] = _build_nc()
    nc = _cache["nc"]

    wq_h = (Wq.reshape(D, N * H) * SCALE).astype(NP_MM)
    wk_h = Wk.reshape(D, N * H).astype(NP_MM)
    wv_h = Wv.reshape(D, N * H).astype(NP_MM)
    bq_h = (bq.reshape(N * H) * SCALE).astype(np.float32)
    bk_h = bk.reshape(N * H).astype(np.float32)

    in_maps = []
    toT_by_b = {}
    for core in range(NCORES):
        b, fh = core // 2, core % 2
        fsl = slice(fh * FS, (fh + 1) * FS)
        if b not in toT_by_b:
            toT_by_b[b] = to_tensor[b].T.astype(NP_MM)
        fromT = from_tensor[b, fsl, :].T.astype(NP_MM)
        adderT = np.ascontiguousarray(
            ((1.0 - attention_mask[b, fsl, :].astype(np.float32))
             * -10000.0).T)
        in_maps.append({
            "fromT": fromT,
            "toT": toT_by_b[b],
            "wq": wq_h, "wk": wk_h, "wv": wv_h,
            "bqv": bq_h, "bkv": bk_h,
            "adderT": adderT,
        })

    _cache["last_in_maps"] = in_maps
    res = run_bass_kernel_spmd(nc, in_maps, list(range(NCORES)))
    _cache["last_result"] = res

    scores = np.empty((B, N, F, T), np.float32)
    context = np.empty((B, F, N, H), np.float32)
    for core in range(NCORES):
        b, fh = core // 2, core % 2
        fsl = slice(fh * FS, (fh + 1) * FS)
        st = res.results[core]["scores_t"]          # [N, T, FS]
        scores[b, :, fsl, :] = st.transpose(0, 2, 1)
        ct = res.results[core]["ctx_t"]             # [N, H+1, FS]
        ctx = ct[:, :H, :] / ct[:, H:H + 1, :]      # [N, H, FS]
        context[b, fsl] = ctx.transpose(2, 0, 1) + bv[None, :, :]
    return context, scores


# revision 6
# speedup vs baseline: 11.2342x; 11.2342x over previous
"""Trainium2 Bass kernel for CustomAttention (B=4, F=T=2048, D=768, N=12, H=64).

Returns (context [B,F,N,H] f32, scores [B,N,F,T] f32) where scores are the
masked, scaled pre-softmax logits.

Sharding: 8 cores = batch(4) x F-half(2). Each core computes, for its
(batch, F-half) slice, all 12 heads:
  - qT [H, F_loc], kT [H, T] projections (head dim on partitions) from
    host-pre-transposed fromT/toT, with 1/sqrt(H) folded into Wq/bq.
  - v [T, H] projections with an appended ones column (v_aug [T, H+1]).
  - per head: scoresT[t, f] = kT.T-style matmul -> +adderT (mask) on DVE
    (this fp32 tile is the scores output, written transposed; host
    transposes back) -> exp on ACT (bf16 probsT) -> context matmul
    lhsT=v_aug, rhs=probsT accumulating over t, giving
    ctx_t [H+1, F_loc]: rows 0..63 = unnormalized context, row 64 = the
    softmax denominator. Host divides, adds bv, reassembles.
"""

import sys

if "/opt/trn_rl_repo" not in sys.path:
    sys.path.insert(0, "/opt/trn_rl_repo")

import numpy as np
import ml_dtypes

import concourse.bass as bass  # noqa: F401  (engine types resolve through nc)
import concourse.mybir as mybir
import concourse.tile as tile
from concourse import bacc
from concourse.bass_utils import run_bass_kernel_spmd

B, F, T, D = 4, 2048, 2048, 768
N, H = 12, 64
NCORES = 8
FS = F // 2          # F rows per core
NPAIR = N // 2       # head pairs (2 heads packed per 128 partitions)
DT6 = D // 128       # contraction tiles over D
TT = T // 128        # t tiles
FC = FS // 512       # 512-wide moving chunks over F_loc
SCALE = 1.0 / 8.0    # 1/sqrt(H)

MM_DT = mybir.dt.bfloat16
NP_MM = ml_dtypes.bfloat16
F32 = mybir.dt.float32

ExpF = mybir.ActivationFunctionType.Exp
CopyF = mybir.ActivationFunctionType.Copy
IdentF = mybir.ActivationFunctionType.Identity

_cache: dict = {}


def _emit_body(nc, tc, io, rep):
    """One full pass of the kernel (projections + scores/context)."""
    r = f"r{rep}"
    with (
        tc.tile_pool(name=f"persist{r}", bufs=1) as pp,
        tc.tile_pool(name=f"consts{r}", bufs=1) as cst,
    ):
        # Persistent SBUF residents.
        qt_all = pp.tile([128, NPAIR, FS], MM_DT, name=f"qt_all{r}")
        kt_all = pp.tile([128, NPAIR, T], MM_DT, name=f"kt_all{r}")
        v_all = pp.tile([128, N, TT, H + 1], MM_DT, name=f"v_all{r}")
        adder_sb = pp.tile([128, TT, FS], F32, name=f"adder_sb{r}")

        bq_sb = cst.tile([128, NPAIR], F32, name=f"bq_sb{r}")
        bk_sb = cst.tile([128, NPAIR], F32, name=f"bk_sb{r}")

        nc.sync.dma_start(
            out=adder_sb,
            in_=io["adderT"].ap().rearrange("(tt p) f -> p tt f", p=128))
        nc.sync.dma_start(
            out=bq_sb, in_=io["bqv"].ap().rearrange("(n p) -> p n", p=128))
        nc.sync.dma_start(
            out=bk_sb, in_=io["bkv"].ap().rearrange("(n p) -> p n", p=128))
        nc.vector.memset(v_all[:, :, :, H:H + 1], 1.0)

        # ---------------- Phase 1: projections ----------------
        with (
            tc.tile_pool(name=f"p1{r}", bufs=1) as p1,
            tc.tile_pool(name=f"ppsum{r}", bufs=4, space="PSUM") as ppsum,
        ):
            from_sb = p1.tile([128, DT6, FS], MM_DT, name=f"from_sb{r}")
            to_sb = p1.tile([128, DT6, T], MM_DT, name=f"to_sb{r}")
            wq_sb = p1.tile([128, DT6, N * H], MM_DT, name=f"wq_sb{r}")
            wk_sb = p1.tile([128, DT6, N * H], MM_DT, name=f"wk_sb{r}")
            wv_sb = p1.tile([128, DT6, N * H], MM_DT, name=f"wv_sb{r}")
            nc.sync.dma_start(
                out=from_sb,
                in_=io["fromT"].ap().rearrange("(n p) f -> p n f", p=128))
            nc.sync.dma_start(
                out=to_sb,
                in_=io["toT"].ap().rearrange("(n p) t -> p n t", p=128))
            nc.sync.dma_start(
                out=wq_sb,
                in_=io["wq"].ap().rearrange("(n p) h -> p n h", p=128))
            nc.sync.dma_start(
                out=wk_sb,
                in_=io["wk"].ap().rearrange("(n p) h -> p n h", p=128))
            nc.sync.dma_start(
                out=wv_sb,
                in_=io["wv"].ap().rearrange("(n p) h -> p n h", p=128))

            # qT / kT: out partitions = 2 heads x 64, moving = F or T.
            for pair in range(NPAIR):
                hs = slice(pair * 128, (pair + 1) * 128)
                for fc in range(FC):
                    ps = ppsum.tile([128, 512], F32, tag="ps", name="ps")
                    for d6 in range(DT6):
                        nc.tensor.matmul(
                            ps, lhsT=wq_sb[:, d6, hs],
                            rhs=from_sb[:, d6, fc * 512:(fc + 1) * 512],
                            start=(d6 == 0), stop=(d6 == DT6 - 1))
                    nc.scalar.activation(
                        qt_all[:, pair, fc * 512:(fc + 1) * 512], ps,
                        IdentF, bias=bq_sb[:, pair:pair + 1])
                for tc4 in range(T // 512):
                    ps = ppsum.tile([128, 512], F32, tag="ps", name="ps")
                    for d6 in range(DT6):
                        nc.tensor.matmul(
                            ps, lhsT=wk_sb[:, d6, hs],
                            rhs=to_sb[:, d6, tc4 * 512:(tc4 + 1) * 512],
                            start=(d6 == 0), stop=(d6 == DT6 - 1))
                    nc.scalar.activation(
                        kt_all[:, pair, tc4 * 512:(tc4 + 1) * 512], ps,
                        IdentF, bias=bk_sb[:, pair:pair + 1])

            # v: out partitions = t rows, moving = 6 heads x 64.
            for tt in range(TT):
                ts_ = slice(tt * 128, (tt + 1) * 128)
                for hg in range(2):
                    ps = ppsum.tile([128, 384], F32, tag="psv", name="psv")
                    for d6 in range(DT6):
                        nc.tensor.matmul(
                            ps, lhsT=to_sb[:, d6, ts_],
                            rhs=wv_sb[:, d6, hg * 384:(hg + 1) * 384],
                            start=(d6 == 0), stop=(d6 == DT6 - 1))
                    for h6 in range(6):
                        h = hg * 6 + h6
                        nc.scalar.activation(
                            v_all[:, h, tt, 0:H],
                            ps[:, h6 * 64:(h6 + 1) * 64], CopyF)

        # ---------------- Phase 2: scores + context ----------------
        with (
            tc.tile_pool(name=f"work{r}", bufs=4) as wk_pool,
            tc.tile_pool(name=f"spsum{r}", bufs=4, space="PSUM") as spsum,
            tc.tile_pool(name=f"cpsum{r}", bufs=2, space="PSUM") as cpsum,
        ):
            for h in range(N):
                pair, sub = h // 2, h % 2
                hp = slice(sub * 64, (sub + 1) * 64)
                ctx_ps = [cpsum.tile([H + 1, 512], F32, tag=f"ctx{fc}",
                                     name=f"ctx_ps{fc}_{h}{r}")
                          for fc in range(FC)]
                for tt in range(TT):
                    sT = wk_pool.tile([128, FS], F32, tag="sT", name="sT")
                    pT = wk_pool.tile([128, FS], MM_DT, tag="pT", name="pT")
                    for fc in range(FC):
                        fsl = slice(fc * 512, (fc + 1) * 512)
                        ps = spsum.tile([128, 512], F32, tag="ps", name="ps")
                        nc.tensor.matmul(
                            ps,
                            lhsT=kt_all[hp, pair, tt * 128:(tt + 1) * 128],
                            rhs=qt_all[hp, pair, fsl],
                            start=True, stop=True)
                        nc.vector.tensor_add(
                            sT[:, fsl], ps, adder_sb[:, tt, fsl])
                        nc.scalar.activation(pT[:, fsl], sT[:, fsl], ExpF)
                        nc.tensor.matmul(
                            ctx_ps[fc], lhsT=v_all[:, h, tt, :],
                            rhs=pT[:, fsl],
                            start=(tt == 0), stop=(tt == TT - 1))
                    nc.sync.dma_start(
                        out=io["scores_t"].ap()[h, tt * 128:(tt + 1) * 128,
                                                :],
                        in_=sT)
                ctx_sb = wk_pool.tile([H + 1, FS], F32, tag="ctx_sb",
                                      name=f"ctx_sb_{h}{r}")
                for fc in range(FC):
                    nc.scalar.activation(
                        ctx_sb[:, fc * 512:(fc + 1) * 512], ctx_ps[fc],
                        CopyF)
                nc.sync.dma_start(out=io["ctx_t"].ap()[h, :, :], in_=ctx_sb)


def _build_nc(repeat=1):
    nc = bacc.Bacc("TRN2", target_bir_lowering=False, debug=False,
                   num_devices=NCORES)
    io = {
        "fromT": nc.dram_tensor("fromT", [D, FS], MM_DT,
                                kind="ExternalInput"),
        "toT": nc.dram_tensor("toT", [D, T], MM_DT, kind="ExternalInput"),
        "wq": nc.dram_tensor("wq", [D, N * H], MM_DT, kind="ExternalInput"),
        "wk": nc.dram_tensor("wk", [D, N * H], MM_DT, kind="ExternalInput"),
        "wv": nc.dram_tensor("wv", [D, N * H], MM_DT, kind="ExternalInput"),
        "bqv": nc.dram_tensor("bqv", [N * H], F32, kind="ExternalInput"),
        "bkv": nc.dram_tensor("bkv", [N * H], F32, kind="ExternalInput"),
        "adderT": nc.dram_tensor("adderT", [T, FS], F32,
                                 kind="ExternalInput"),
        "scores_t": nc.dram_tensor("scores_t", [N, T, FS], F32,
                                   kind="ExternalOutput"),
        "ctx_t": nc.dram_tensor("ctx_t", [N, H + 1, FS], F32,
                                kind="ExternalOutput"),
    }

    with tile.TileContext(nc) as tc:
        for rep in range(repeat):
            _emit_body(nc, tc, io, rep)

    nc.compile()
    return nc


def kernel(from_tensor, to_tensor, attention_mask, Wq, Wk, Wv, bq, bk, bv):
    from_tensor = np.asarray(from_tensor, dtype=np.float32)
    to_tensor = np.asarray(to_tensor, dtype=np.float32)
    attention_mask = np.asarray(attention_mask)
    Wq = np.asarray(Wq, dtype=np.float32)
    Wk = np.asarray(Wk, dtype=np.float32)
    Wv = np.asarray(Wv, dtype=np.float32)
    bq = np.asarray(bq, dtype=np.float32)
    bk = np.asarray(bk, dtype=np.float32)
    bv = np.asarray(bv, dtype=np.float32)

    if "nc" not in _cache:
        _cache["nc"] = _build_nc()
    nc = _cache["nc"]

    wq_h = (Wq.reshape(D, N * H) * SCALE).astype(NP_MM)
    wk_h = Wk.reshape(D, N * H).astype(NP_MM)
    wv_h = Wv.reshape(D, N * H).astype(NP_MM)
    bq_h = (bq.reshape(N * H) * SCALE).astype(np.float32)
    bk_h = bk.reshape(N * H).astype(np.float32)

    in_maps = []
    toT_by_b = {}
    for core in range(NCORES):
        b, fh = core // 2, core % 2
        fsl = slice(fh * FS, (fh + 1) * FS)
        if b not in toT_by_b:
            toT_by_b[b] = to_tensor[b].T.astype(NP_MM)
        fromT = from_tensor[b, fsl, :].T.astype(NP_MM)
        adderT = np.ascontiguousarray(
            ((1.0 - attention_mask[b, fsl, :].astype(np.float32))
             * -10000.0).T)
        in_maps.append({
            "fromT": fromT,
            "toT": toT_by_b[b],
            "wq": wq_h, "wk": wk_h, "wv": wv_h,
            "bqv": bq_h, "bkv": bk_h,
            "adderT": adderT,
        })

    _cache["last_in_maps"] = in_maps
    res = run_bass_kernel_spmd(nc, in_maps, list(range(NCORES)))
    _cache["last_result"] = res

    scores = np.empty((B, N, F, T), np.float32)
    context = np.empty((B, F, N, H), np.float32)
    for core in range(NCORES):
        b, fh = core // 2, core % 2
        fsl = slice(fh * FS, (fh + 1) * FS)
        st = res.results[core]["scores_t"]          # [N, T, FS]
        scores[b, :, fsl, :] = st.transpose(0, 2, 1)
        ct = res.results[core]["ctx_t"]             # [N, H+1, FS]
        ctx = ct[:, :H, :] / ct[:, H:H + 1, :]      # [N, H, FS]
        context[b, fsl] = ctx.transpose(2, 0, 1) + bv[None, :, :]
    return context, scores


# revision 7
# speedup vs baseline: 20.1601x; 1.7945x over previous
"""Trainium2 Bass kernel for CustomAttention (B=4, F=T=2048, D=768, N=12, H=64).

Returns (context [B,F,N,H] f32, scores [B,N,F,T] f32) where scores are the
masked, scaled pre-softmax logits.

Sharding: 8 cores = batch(4) x F-half(2). Each core computes, for its
(batch, F-half) slice, all 12 heads:
  - qT [H, F_loc], kT [H, T] projections (head dim on partitions) from
    host-pre-transposed fromT/toT, with 1/sqrt(H) folded into Wq/bq.
  - v [T, H] projections with an appended ones column (v_aug [T, H+1]).
  - per head: scoresT[t, f] = kT.T-style matmul -> +adderT (mask) on DVE
    (this fp32 tile is the scores output, written transposed; host
    transposes back) -> exp on ACT (bf16 probsT) -> context matmul
    lhsT=v_aug, rhs=probsT accumulating over t, giving
    ctx_t [H+1, F_loc]: rows 0..63 = unnormalized context, row 64 = the
    softmax denominator. Host divides, adds bv, reassembles.
"""

import sys

if "/opt/trn_rl_repo" not in sys.path:
    sys.path.insert(0, "/opt/trn_rl_repo")

import numpy as np
import ml_dtypes

import concourse.bass as bass  # noqa: F401  (engine types resolve through nc)
import concourse.mybir as mybir
import concourse.tile as tile
from concourse import bacc
from concourse.bass_utils import run_bass_kernel_spmd

B, F, T, D = 4, 2048, 2048, 768
N, H = 12, 64
NCORES = 8
FS = F // 2          # F rows per core
NPAIR = N // 2       # head pairs (2 heads packed per 128 partitions)
DT6 = D // 128       # contraction tiles over D
TT = T // 128        # t tiles
FC = FS // 512       # 512-wide moving chunks over F_loc
SCALE = 1.0 / 8.0    # 1/sqrt(H)

MM_DT = mybir.dt.bfloat16
NP_MM = ml_dtypes.bfloat16
F32 = mybir.dt.float32

ExpF = mybir.ActivationFunctionType.Exp
CopyF = mybir.ActivationFunctionType.Copy
IdentF = mybir.ActivationFunctionType.Identity

_cache: dict = {}


def _emit_body(nc, tc, io, rep):
    """One full pass of the kernel (projections + scores/context)."""
    r = f"r{rep}"
    with (
        tc.tile_pool(name=f"persist{r}", bufs=1) as pp,
        tc.tile_pool(name=f"consts{r}", bufs=1) as cst,
    ):
        # Persistent SBUF residents.
        qt_all = pp.tile([128, NPAIR, FS], MM_DT, name=f"qt_all{r}")
        kt_all = pp.tile([128, NPAIR, T], MM_DT, name=f"kt_all{r}")
        v_all = pp.tile([128, N, TT, H + 1], MM_DT, name=f"v_all{r}")
        adder_sb = pp.tile([128, TT, FS], F32, name=f"adder_sb{r}")

        bq_sb = cst.tile([128, NPAIR], F32, name=f"bq_sb{r}")
        bk_sb = cst.tile([128, NPAIR], F32, name=f"bk_sb{r}")

        nc.sync.dma_start(
            out=adder_sb,
            in_=io["adderT"].ap().rearrange("(tt p) f -> p tt f", p=128))
        nc.sync.dma_start(
            out=bq_sb, in_=io["bqv"].ap().rearrange("(n p) -> p n", p=128))
        nc.sync.dma_start(
            out=bk_sb, in_=io["bkv"].ap().rearrange("(n p) -> p n", p=128))
        nc.vector.memset(v_all[:, :, :, H:H + 1], 1.0)

        # ---------------- Phase 1: projections ----------------
        with (
            tc.tile_pool(name=f"p1{r}", bufs=1) as p1,
            tc.tile_pool(name=f"ppsum{r}", bufs=4, space="PSUM") as ppsum,
        ):
            from_sb = p1.tile([128, DT6, FS], MM_DT, name=f"from_sb{r}")
            to_sb = p1.tile([128, DT6, T], MM_DT, name=f"to_sb{r}")
            wq_sb = p1.tile([128, DT6, N * H], MM_DT, name=f"wq_sb{r}")
            wk_sb = p1.tile([128, DT6, N * H], MM_DT, name=f"wk_sb{r}")
            wv_sb = p1.tile([128, DT6, N * H], MM_DT, name=f"wv_sb{r}")
            nc.sync.dma_start(
                out=from_sb,
                in_=io["fromT"].ap().rearrange("(n p) f -> p n f", p=128))
            nc.sync.dma_start(
                out=to_sb,
                in_=io["toT"].ap().rearrange("(n p) t -> p n t", p=128))
            nc.sync.dma_start(
                out=wq_sb,
                in_=io["wq"].ap().rearrange("(n p) h -> p n h", p=128))
            nc.sync.dma_start(
                out=wk_sb,
                in_=io["wk"].ap().rearrange("(n p) h -> p n h", p=128))
            nc.sync.dma_start(
                out=wv_sb,
                in_=io["wv"].ap().rearrange("(n p) h -> p n h", p=128))

            # qT / kT: out partitions = 2 heads x 64, moving = F or T.
            for pair in range(NPAIR):
                hs = slice(pair * 128, (pair + 1) * 128)
                for fc in range(FC):
                    ps = ppsum.tile([128, 512], F32, tag="ps", name="ps")
                    for d6 in range(DT6):
                        nc.tensor.matmul(
                            ps, lhsT=wq_sb[:, d6, hs],
                            rhs=from_sb[:, d6, fc * 512:(fc + 1) * 512],
                            start=(d6 == 0), stop=(d6 == DT6 - 1))
                    nc.scalar.activation(
                        qt_all[:, pair, fc * 512:(fc + 1) * 512], ps,
                        IdentF, bias=bq_sb[:, pair:pair + 1])
                for tc4 in range(T // 512):
                    ps = ppsum.tile([128, 512], F32, tag="ps", name="ps")
                    for d6 in range(DT6):
                        nc.tensor.matmul(
                            ps, lhsT=wk_sb[:, d6, hs],
                            rhs=to_sb[:, d6, tc4 * 512:(tc4 + 1) * 512],
                            start=(d6 == 0), stop=(d6 == DT6 - 1))
                    nc.scalar.activation(
                        kt_all[:, pair, tc4 * 512:(tc4 + 1) * 512], ps,
                        IdentF, bias=bk_sb[:, pair:pair + 1])

            # v: out partitions = t rows, moving = 6 heads x 64.
            for tt in range(TT):
                ts_ = slice(tt * 128, (tt + 1) * 128)
                for hg in range(2):
                    ps = ppsum.tile([128, 384], F32, tag="psv", name="psv")
                    for d6 in range(DT6):
                        nc.tensor.matmul(
                            ps, lhsT=to_sb[:, d6, ts_],
                            rhs=wv_sb[:, d6, hg * 384:(hg + 1) * 384],
                            start=(d6 == 0), stop=(d6 == DT6 - 1))
                    for h6 in range(6):
                        h = hg * 6 + h6
                        nc.scalar.activation(
                            v_all[:, h, tt, 0:H],
                            ps[:, h6 * 64:(h6 + 1) * 64], CopyF)

        # ---------------- Phase 2: scores + context ----------------
        with (
            tc.tile_pool(name=f"work{r}", bufs=4) as wk_pool,
            tc.tile_pool(name=f"spsum{r}", bufs=4, space="PSUM") as spsum,
            tc.tile_pool(name=f"cpsum{r}", bufs=2, space="PSUM") as cpsum,
        ):
            for h in range(N):
                pair, sub = h // 2, h % 2
                hp = slice(sub * 64, (sub + 1) * 64)
                ctx_ps = [cpsum.tile([H + 1, 512], F32, tag=f"ctx{fc}",
                                     name=f"ctx_ps{fc}_{h}{r}")
                          for fc in range(FC)]
                for tt in range(TT):
                    sT = wk_pool.tile([128, FS], F32, tag="sT", name="sT")
                    pT = wk_pool.tile([128, FS], MM_DT, tag="pT", name="pT")
                    for fc in range(FC):
                        fsl = slice(fc * 512, (fc + 1) * 512)
                        ps = spsum.tile([128, 512], F32, tag="ps", name="ps")
                        nc.tensor.matmul(
                            ps,
                            lhsT=kt_all[hp, pair, tt * 128:(tt + 1) * 128],
                            rhs=qt_all[hp, pair, fsl],
                            start=True, stop=True)
                        nc.vector.tensor_add(
                            sT[:, fsl], ps, adder_sb[:, tt, fsl])
                        nc.scalar.activation(pT[:, fsl], sT[:, fsl], ExpF)
                        nc.tensor.matmul(
                            ctx_ps[fc], lhsT=v_all[:, h, tt, :],
                            rhs=pT[:, fsl],
                            start=(tt == 0), stop=(tt == TT - 1))
                    nc.sync.dma_start(
                        out=io["scores_t"].ap()[h, tt * 128:(tt + 1) * 128,
                                                :],
                        in_=sT)
                ctx_sb = wk_pool.tile([H + 1, FS], F32, tag="ctx_sb",
                                      name=f"ctx_sb_{h}{r}")
                for fc in range(FC):
                    nc.scalar.activation(
                        ctx_sb[:, fc * 512:(fc + 1) * 512], ctx_ps[fc],
                        CopyF)
                nc.sync.dma_start(out=io["ctx_t"].ap()[h, :, :], in_=ctx_sb)


def _build_nc(repeat=1, loop_repeat=1):
    nc = bacc.Bacc("TRN2", target_bir_lowering=False, debug=False,
                   num_devices=NCORES)
    io = {
        "fromT": nc.dram_tensor("fromT", [D, FS], MM_DT,
                                kind="ExternalInput"),
        "toT": nc.dram_tensor("toT", [D, T], MM_DT, kind="ExternalInput"),
        "wq": nc.dram_tensor("wq", [D, N * H], MM_DT, kind="ExternalInput"),
        "wk": nc.dram_tensor("wk", [D, N * H], MM_DT, kind="ExternalInput"),
        "wv": nc.dram_tensor("wv", [D, N * H], MM_DT, kind="ExternalInput"),
        "bqv": nc.dram_tensor("bqv", [N * H], F32, kind="ExternalInput"),
        "bkv": nc.dram_tensor("bkv", [N * H], F32, kind="ExternalInput"),
        "adderT": nc.dram_tensor("adderT", [T, FS], F32,
                                 kind="ExternalInput"),
        "scores_t": nc.dram_tensor("scores_t", [N, T, FS], F32,
                                   kind="ExternalOutput"),
        "ctx_t": nc.dram_tensor("ctx_t", [N, H + 1, FS], F32,
                                kind="ExternalOutput"),
    }

    with tile.TileContext(nc) as tc:
        if loop_repeat > 1:
            with tc.For_i(0, loop_repeat, 1):
                _emit_body(nc, tc, io, 0)
        else:
            for rep in range(repeat):
                _emit_body(nc, tc, io, rep)

    nc.compile()
    return nc


def kernel(from_tensor, to_tensor, attention_mask, Wq, Wk, Wv, bq, bk, bv):
    from_tensor = np.asarray(from_tensor, dtype=np.float32)
    to_tensor = np.asarray(to_tensor, dtype=np.float32)
    attention_mask = np.asarray(attention_mask)
    Wq = np.asarray(Wq, dtype=np.float32)
    Wk = np.asarray(Wk, dtype=np.float32)
    Wv = np.asarray(Wv, dtype=np.float32)
    bq = np.asarray(bq, dtype=np.float32)
    bk = np.asarray(bk, dtype=np.float32)
    bv = np.asarray(bv, dtype=np.float32)

    if "nc" not in _cache:
        _cache["nc"] = _build_nc()
    nc = _cache["nc"]

    wq_h = (Wq.reshape(D, N * H) * SCALE).astype(NP_MM)
    wk_h = Wk.reshape(D, N * H).astype(NP_MM)
    wv_h = Wv.reshape(D, N * H).astype(NP_MM)
    bq_h = (bq.reshape(N * H) * SCALE).astype(np.float32)
    bk_h = bk.reshape(N * H).astype(np.float32)

    in_maps = []
    toT_by_b = {}
    for core in range(NCORES):
        b, fh = core // 2, core % 2
        fsl = slice(fh * FS, (fh + 1) * FS)
        if b not in toT_by_b:
            toT_by_b[b] = to_tensor[b].T.astype(NP_MM)
        fromT = from_tensor[b, fsl, :].T.astype(NP_MM)
        adderT = np.ascontiguousarray(
            ((1.0 - attention_mask[b, fsl, :].astype(np.float32))
             * -10000.0).T)
        in_maps.append({
            "fromT": fromT,
            "toT": toT_by_b[b],
            "wq": wq_h, "wk": wk_h, "wv": wv_h,
            "bqv": bq_h, "bkv": bk_h,
            "adderT": adderT,
        })

    _cache["last_in_maps"] = in_maps
    res = run_bass_kernel_spmd(nc, in_maps, list(range(NCORES)))
    _cache["last_result"] = res

    scores = np.empty((B, N, F, T), np.float32)
    context = np.empty((B, F, N, H), np.float32)
    for core in range(NCORES):
        b, fh = core // 2, core % 2
        fsl = slice(fh * FS, (fh + 1) * FS)
        st = res.results[core]["scores_t"]          # [N, T, FS]
        scores[b, :, fsl, :] = st.transpose(0, 2, 1)
        ct = res.results[core]["ctx_t"]             # [N, H+1, FS]
        ctx = ct[:, :H, :] / ct[:, H:H + 1, :]      # [N, H, FS]
        context[b, fsl] = ctx.transpose(2, 0, 1) + bv[None, :, :]
    return context, scores


# revision 8
# speedup vs baseline: 39.5823x; 1.9634x over previous
"""Trainium2 Bass kernel for CustomAttention (B=4, F=T=2048, D=768, N=12, H=64).

Returns (context [B,F,N,H] f32, scores [B,N,F,T] f32) where scores are the
masked, scaled pre-softmax logits.

Sharding: 8 cores = batch(4) x F-half(2). Each core computes, for its
(batch, F-half) slice, all 12 heads:
  - qT [H, F_loc], kT [H, T] projections (head dim on partitions) from
    host-pre-transposed fromT/toT, with 1/sqrt(H) folded into Wq/bq.
  - v [T, H] projections with an appended ones column (v_aug [T, H+1]).
  - per head: scoresT[t, f] = kT/qT matmul -> +adderT (fp16 mask adder,
    exact for 0/-10000) on DVE (fp32 tile = scores output, written
    transposed; host transposes back) -> exp on ACT (bf16 probsT) ->
    context matmul lhsT=v_aug, rhs=probsT accumulating over t:
    ctx_t [H+1, F_loc] rows 0..63 = unnormalized context, row 64 = softmax
    denominator. Host divides, adds bv, reassembles.

Projections are interleaved with the per-head attention loop so the
dominant scores-output DMA stream starts early.
"""

import sys

if "/opt/trn_rl_repo" not in sys.path:
    sys.path.insert(0, "/opt/trn_rl_repo")

import numpy as np
import ml_dtypes

import concourse.bass as bass  # noqa: F401
import concourse.mybir as mybir
import concourse.tile as tile
from concourse import bacc
from concourse.bass_utils import run_bass_kernel_spmd

B, F, T, D = 4, 2048, 2048, 768
N, H = 12, 64
NCORES = 8
FS = F // 2          # F rows per core
NPAIR = N // 2       # head pairs (2 heads packed per 128 partitions)
DT6 = D // 128       # contraction tiles over D
TT = T // 128        # t tiles
FC = FS // 512       # 512-wide moving chunks over F_loc
SCALE = 1.0 / 8.0    # 1/sqrt(H)

MM_DT = mybir.dt.bfloat16
NP_MM = ml_dtypes.bfloat16
F32 = mybir.dt.float32
F16 = mybir.dt.float16

ExpF = mybir.ActivationFunctionType.Exp
CopyF = mybir.ActivationFunctionType.Copy
IdentF = mybir.ActivationFunctionType.Identity

_cache: dict = {}


def _emit_body(nc, tc, io, rep):
    """One full pass of the kernel (projections interleaved with heads)."""
    r = f"r{rep}"
    with (
        tc.tile_pool(name=f"persist{r}", bufs=1) as pp,
        tc.tile_pool(name=f"p1{r}", bufs=1) as p1,
        tc.tile_pool(name=f"work{r}", bufs=1) as wk_pool,
        tc.tile_pool(name=f"psum{r}", bufs=1, space="PSUM") as psum,
    ):
        qt_all = pp.tile([128, NPAIR, FS], MM_DT, name=f"qt_all{r}")
        kt_all = pp.tile([128, NPAIR, T], MM_DT, name=f"kt_all{r}")
        v_all = pp.tile([128, N, TT, H + 1], MM_DT, name=f"v_all{r}")
        adder_sb = pp.tile([128, TT, FS], F16, name=f"adder_sb{r}")
        bq_sb = pp.tile([128, NPAIR], F32, name=f"bq_sb{r}")
        bk_sb = pp.tile([128, NPAIR], F32, name=f"bk_sb{r}")

        from_sb = p1.tile([128, DT6, FS], MM_DT, name=f"from_sb{r}")
        to_sb = p1.tile([128, DT6, T], MM_DT, name=f"to_sb{r}")
        wq_sb = p1.tile([128, DT6, N * H], MM_DT, name=f"wq_sb{r}")
        wk_sb = p1.tile([128, DT6, N * H], MM_DT, name=f"wk_sb{r}")
        wv_sb = p1.tile([128, DT6, N * H], MM_DT, name=f"wv_sb{r}")

        nc.sync.dma_start(
            out=to_sb, in_=io["toT"].ap().rearrange("(n p) t -> p n t",
                                                    p=128))
        nc.sync.dma_start(
            out=from_sb, in_=io["fromT"].ap().rearrange("(n p) f -> p n f",
                                                        p=128))
        nc.sync.dma_start(
            out=wq_sb, in_=io["wq"].ap().rearrange("(n p) h -> p n h",
                                                   p=128))
        nc.sync.dma_start(
            out=wk_sb, in_=io["wk"].ap().rearrange("(n p) h -> p n h",
                                                   p=128))
        nc.sync.dma_start(
            out=wv_sb, in_=io["wv"].ap().rearrange("(n p) h -> p n h",
                                                   p=128))
        nc.sync.dma_start(
            out=adder_sb, in_=io["adderT"].ap().rearrange(
                "(tt p) f -> p tt f", p=128))
        nc.sync.dma_start(
            out=bq_sb, in_=io["bqv"].ap().rearrange("(n p) -> p n", p=128))
        nc.sync.dma_start(
            out=bk_sb, in_=io["bkv"].ap().rearrange("(n p) -> p n", p=128))
        nc.vector.memset(v_all[:, :, :, H:H + 1], 1.0)

        def proj_v(hg):
            # v for heads hg*6 .. hg*6+5, all t tiles
            for tt in range(TT):
                ts_ = slice(tt * 128, (tt + 1) * 128)
                ps = psum.tile([128, 384], F32, tag="proj", name="psv",
                               bufs=2)
                for d6 in range(DT6):
                    nc.tensor.matmul(
                        ps, lhsT=to_sb[:, d6, ts_],
                        rhs=wv_sb[:, d6, hg * 384:(hg + 1) * 384],
                        start=(d6 == 0), stop=(d6 == DT6 - 1))
                nc.scalar.activation(
                    v_all[:, hg * 6:(hg + 1) * 6, tt, 0:H],
                    ps.rearrange("p (a b) -> p a b", a=6), CopyF)

        def proj_qk(pair):
            hs = slice(pair * 128, (pair + 1) * 128)
            for fc in range(FC):
                ps = psum.tile([128, 512], F32, tag="proj", name="psq",
                               bufs=2)
                for d6 in range(DT6):
                    nc.tensor.matmul(
                        ps, lhsT=wq_sb[:, d6, hs],
                        rhs=from_sb[:, d6, fc * 512:(fc + 1) * 512],
                        start=(d6 == 0), stop=(d6 == DT6 - 1))
                nc.scalar.activation(
                    qt_all[:, pair, fc * 512:(fc + 1) * 512], ps,
                    IdentF, bias=bq_sb[:, pair:pair + 1])
            for tc4 in range(T // 512):
                ps = psum.tile([128, 512], F32, tag="proj", name="psk",
                               bufs=2)
                for d6 in range(DT6):
                    nc.tensor.matmul(
                        ps, lhsT=wk_sb[:, d6, hs],
                        rhs=to_sb[:, d6, tc4 * 512:(tc4 + 1) * 512],
                        start=(d6 == 0), stop=(d6 == DT6 - 1))
                nc.scalar.activation(
                    kt_all[:, pair, tc4 * 512:(tc4 + 1) * 512], ps,
                    IdentF, bias=bk_sb[:, pair:pair + 1])

        def head_body(h):
            pair, sub = h // 2, h % 2
            hp = slice(sub * 64, (sub + 1) * 64)
            ctx_ps = [psum.tile([H + 1, 512], F32, tag=f"ctx{fc}",
                                name=f"ctx_ps{fc}_{h}{r}", bufs=1)
                      for fc in range(FC)]
            for tt in range(TT):
                sT = wk_pool.tile([128, FS], F32, tag="sT", name="sT",
                                  bufs=3)
                pT = wk_pool.tile([128, FS], MM_DT, tag="pT", name="pT",
                                  bufs=3)
                sps = psum.tile([128, FC, 512], F32, tag="sps", name="sps",
                                bufs=2)
                for fc in range(FC):
                    nc.tensor.matmul(
                        sps[:, fc, :],
                        lhsT=kt_all[hp, pair, tt * 128:(tt + 1) * 128],
                        rhs=qt_all[hp, pair, fc * 512:(fc + 1) * 512],
                        start=True, stop=True)
                nc.vector.tensor_add(
                    sT.rearrange("p (a b) -> p a b", a=FC), sps,
                    adder_sb[:, tt, :].rearrange("p (a b) -> p a b", a=FC))
                nc.scalar.activation(pT, sT, ExpF)
                for fc in range(FC):
                    nc.tensor.matmul(
                        ctx_ps[fc], lhsT=v_all[:, h, tt, :],
                        rhs=pT[:, fc * 512:(fc + 1) * 512],
                        start=(tt == 0), stop=(tt == TT - 1))
                nc.sync.dma_start(
                    out=io["scores_t"].ap()[h, tt * 128:(tt + 1) * 128, :],
                    in_=sT)
            ctx_sb = wk_pool.tile([H + 1, FS], F32, tag="ctx_sb",
                                  name=f"ctx_sb_{h}{r}", bufs=2)
            for fc in range(FC):
                nc.scalar.activation(
                    ctx_sb[:, fc * 512:(fc + 1) * 512], ctx_ps[fc], CopyF)
            nc.sync.dma_start(out=io["ctx_t"].ap()[h, :, :], in_=ctx_sb)

        for pair in range(NPAIR):
            if pair % 3 == 0:
                proj_v(pair // 3)
            proj_qk(pair)
            head_body(2 * pair)
            head_body(2 * pair + 1)


def _build_nc(repeat=1, loop_repeat=1):
    nc = bacc.Bacc("TRN2", target_bir_lowering=False, debug=False,
                   num_devices=NCORES)
    io = {
        "fromT": nc.dram_tensor("fromT", [D, FS], MM_DT,
                                kind="ExternalInput"),
        "toT": nc.dram_tensor("toT", [D, T], MM_DT, kind="ExternalInput"),
        "wq": nc.dram_tensor("wq", [D, N * H], MM_DT, kind="ExternalInput"),
        "wk": nc.dram_tensor("wk", [D, N * H], MM_DT, kind="ExternalInput"),
        "wv": nc.dram_tensor("wv", [D, N * H], MM_DT, kind="ExternalInput"),
        "bqv": nc.dram_tensor("bqv", [N * H], F32, kind="ExternalInput"),
        "bkv": nc.dram_tensor("bkv", [N * H], F32, kind="ExternalInput"),
        "adderT": nc.dram_tensor("adderT", [T, FS], F16,
                                 kind="ExternalInput"),
        "scores_t": nc.dram_tensor("scores_t", [N, T, FS], F32,
                                   kind="ExternalOutput"),
        "ctx_t": nc.dram_tensor("ctx_t", [N, H + 1, FS], F32,
                                kind="ExternalOutput"),
    }

    with tile.TileContext(nc) as tc:
        if loop_repeat > 1:
            with tc.For_i(0, loop_repeat, 1):
                _emit_body(nc, tc, io, 0)
        else:
            for rep in range(repeat):
                _emit_body(nc, tc, io, rep)

    nc.compile()
    return nc


def kernel(from_tensor, to_tensor, attention_mask, Wq, Wk, Wv, bq, bk, bv):
    from_tensor = np.asarray(from_tensor, dtype=np.float32)
    to_tensor = np.asarray(to_tensor, dtype=np.float32)
    attention_mask = np.asarray(attention_mask)
    Wq = np.asarray(Wq, dtype=np.float32)
    Wk = np.asarray(Wk, dtype=np.float32)
    Wv = np.asarray(Wv, dtype=np.float32)
    bq = np.asarray(bq, dtype=np.float32)
    bk = np.asarray(bk, dtype=np.float32)
    bv = np.asarray(bv, dtype=np.float32)

    if "nc" not in _cache:
        _cache["nc"] = _build_nc()
    nc = _cache["nc"]

    wq_h = (Wq.reshape(D, N * H) * SCALE).astype(NP_MM)
    wk_h = Wk.reshape(D, N * H).astype(NP_MM)
    wv_h = Wv.reshape(D, N * H).astype(NP_MM)
    bq_h = (bq.reshape(N * H) * SCALE).astype(np.float32)
    bk_h = bk.reshape(N * H).astype(np.float32)

    in_maps = []
    toT_by_b = {}
    for core in range(NCORES):
        b, fh = core // 2, core % 2
        fsl = slice(fh * FS, (fh + 1) * FS)
        if b not in toT_by_b:
            toT_by_b[b] = to_tensor[b].T.astype(NP_MM)
        fromT = from_tensor[b, fsl, :].T.astype(NP_MM)
        adderT = np.ascontiguousarray(
            ((1.0 - attention_mask[b, fsl, :].astype(np.float32))
             * -10000.0).T).astype(np.float16)
        in_maps.append({
            "fromT": fromT,
            "toT": toT_by_b[b],
            "wq": wq_h, "wk": wk_h, "wv": wv_h,
            "bqv": bq_h, "bkv": bk_h,
            "adderT": adderT,
        })

    _cache["last_in_maps"] = in_maps
    res = run_bass_kernel_spmd(nc, in_maps, list(range(NCORES)))
    _cache["last_result"] = res

    scores = np.empty((B, N, F, T), np.float32)
    context = np.empty((B, F, N, H), np.float32)
    for core in range(NCORES):
        b, fh = core // 2, core % 2
        fsl = slice(fh * FS, (fh + 1) * FS)
        st = res.results[core]["scores_t"]          # [N, T, FS]
        scores[b, :, fsl, :] = st.transpose(0, 2, 1)
        ct = res.results[core]["ctx_t"]             # [N, H+1, FS]
        ctx = ct[:, :H, :] / ct[:, H:H + 1, :]      # [N, H, FS]
        context[b, fsl] = ctx.transpose(2, 0, 1) + bv[None, :, :]
    return context, scores
